# revision 46
# baseline (speedup 1.0000x reference)
"""Multi-head cross-attention (MHAForCrossFusion) on 8 Trainium2 cores.

Strategy: tensor-parallel over heads. Each core owns 2 of the 16 heads
(CW=128 projection features): column slices of Wq/Wk/Wv, row slice of Wo.
q/k/v replicated; each core writes full-shape PER-HEAD partials of the
output projection plus softmax denominator rows; the host divides each
head's partial by its denominators and sums all 16 (+ bo).

Design notes (vs the fp32 v1 baseline, 3.6x faster):
 - all matmuls bf16 (1 PE cycle/col vs 4 for fp32); fp32 PSUM accum
 - q/k/v transposed, bf16-cast and retiled on the HOST: each projection
   input tile [128, DC*512] is one contiguous DRAM block (8KB
   descriptors), and the device does no input transposes at all
 - single flat software pipeline over all 64 (batch, chunk, key-pair)
   steps: scores -> exp -> ctx each lag one step; normalize/out-proj of
   a chunk are deferred 2-4 steps into the next chunk; projections run
   as need-ordered "filler" units inside the same stream so the PE
   never idles long enough for the HAM clock-gate to re-throttle
 - scores: 2 heads row-packed via tile_position run concurrently; two
   key tiles share one [128,1024] psum pair so exp is one big ACT
 - softmax denominator rides row 64 of the ctx accumulation via an
   appended ones column in vma; the denominators are shipped to the
   HOST (den_d) and the division happens there per head: out =
   sum_h out_h / den_h commutes with the per-head out-projection, so
   the device never computes a reciprocal at all
 - out-projection emits per-head partials: two concurrent row-packed
   K=64 matmuls (tile_position (0,0)/(64,0)) into the two "w" psum
   banks; psum -> bf16 sbuf (DVE) -> DMA to out_p0/out_p1
 - PSUM budget exactly 16KB/partition: scores 2x[128,1024] + ctx
   2x[128,512] + shared proj/outproj/transpose slots 2x[128,512]
"""

import numpy as np
from ml_dtypes import bfloat16

import concourse.bass as bass
import concourse.mybir as mybir
import concourse.tile as tile
from concourse import bass_utils
from concourse.masks import make_identity

N_CORES = 8
B, L, D = 2, 2048, 1024
T = B * L  # 4096 flattened tokens; batches are disjoint 2048-token ranges
NH, HD = 16, 64
CW = (NH // N_CORES) * HD  # 128 features per core (2 heads)
DC = D // 128  # 8 contraction tiles for the projections
NBLK = T // 128  # 32 token blocks for vma
SCALE = 1.0 / np.sqrt(HD)

F32 = mybir.dt.float32
BF16 = mybir.dt.bfloat16


def _r(ap):
    return ap.bitcast(mybir.dt.float32r)


def _split_matmul_waits(nc):
    """fp32/fp32r matmuls lower to a self-loading LDW whose ISA struct has a
    single sem-wait slot (HWDGE DMA likewise); walrus rejects >1 wait. Move
    extra waits onto same-engine NoOps inserted right before the matmul
    (program order on the sequencer preserves the happens-before)."""
    for f in nc.m.functions:
        for bb in f.blocks:
            insts = list(bb.instructions)
            out = []
            for inst in insts:
                si = inst.sync_info
                if si is not None and len(si.on_wait) > 1:
                    for w in si.on_wait[:-1]:
                        nop = mybir.InstNoOp(
                            name=nc.get_next_instruction_name(),
                            ins=[],
                            outs=[],
                            engine=inst.engine,
                            bass_nofuse=True,
                        )
                        nop.sync_info = mybir.SyncInfo(on_wait=[w], on_update=[])
                        out.append(nop)
                    inst.sync_info = mybir.SyncInfo(
                        on_wait=[si.on_wait[-1]], on_update=si.on_update
                    )
                out.append(inst)
            if len(out) != len(insts):
                bb.instructions = out
    return nc


def build_nc():
    nc = bass.Bass("TRN2", target_bir_lowering=False, debug=False)

    # host-retiled: [B*NTI tiles, 128 partitions, DC*512] — each (tile,
    # partition) row is 8KB contiguous, so DMA descriptors are 8KB
    qT = nc.dram_tensor("qT", [T // 512, 128, DC * 512], BF16, kind="ExternalInput").ap()
    kT = nc.dram_tensor("kT", [T // 512, 128, DC * 512], BF16, kind="ExternalInput").ap()
    vT = nc.dram_tensor("vT", [T // 512, 128, DC * 512], BF16, kind="ExternalInput").ap()
    # weights host-swizzled to [128, DC*CW] so the DMA is contiguous
    wq = nc.dram_tensor("wq", [128, DC * CW], BF16, kind="ExternalInput").ap()
    wk = nc.dram_tensor("wk", [128, DC * CW], BF16, kind="ExternalInput").ap()
    wv = nc.dram_tensor("wv", [128, DC * CW], BF16, kind="ExternalInput").ap()
    wot = nc.dram_tensor("wot", [CW, D], BF16, kind="ExternalInput").ap()
    bq = nc.dram_tensor("bq", [CW, 1], F32, kind="ExternalInput").ap()
    bk = nc.dram_tensor("bk", [CW, 1], F32, kind="ExternalInput").ap()
    bv = nc.dram_tensor("bv", [CW, 1], F32, kind="ExternalInput").ap()
    out_p0 = nc.dram_tensor("out_p0", [T, D], BF16, kind="ExternalOutput").ap()
    out_p1 = nc.dram_tensor("out_p1", [T, D], BF16, kind="ExternalOutput").ap()
    den_d = nc.dram_tensor("den_d", [B * (L // 512) * 2, 512], F32, kind="ExternalOutput").ap()

    with tile.TileContext(nc) as tc:
        with (
            tc.tile_pool(name="singles", bufs=1) as singles,
            tc.tile_pool(name="acts", bufs=1) as acts,
            tc.tile_pool(name="slab", bufs=12) as slab_pool,
            tc.tile_pool(name="vmf", bufs=2) as vmf_pool,
            tc.tile_pool(name="es", bufs=6) as es_pool,
            tc.tile_pool(name="small", bufs=2) as small,
            tc.tile_pool(name="ob", bufs=3) as ob_pool,
            tc.tile_pool(name="pp_sp", bufs=2, space="PSUM") as pp_sp,
            tc.tile_pool(name="pp_ctx", bufs=2, space="PSUM") as pp_ctx,
            tc.tile_pool(name="pp_w", bufs=2, space="PSUM") as pp_w,
        ):
            ident = singles.tile([128, 128], F32)
            make_identity(nc, ident)

            def emit_warm(n):
                """Back-to-back identity transposes: keeps the PE HAM
                activity monitor at K=8/8 through DMA-paced stretches."""
                wps = pp_w.tile([128, 512], F32, tag="w", name="wps")
                for _ in range(n):
                    nc.tensor.transpose(wps[:, 0:128], ident, ident)

            w_sb = {}
            for name, dram in (("wk", wk), ("wq", wq), ("wv", wv)):
                w = singles.tile([128, DC, CW], BF16, name=name + "_sb")
                nc.sync.dma_start(w.rearrange("p c h -> p (c h)"), dram)
                w_sb[name] = w
            wot_sb = singles.tile([CW, D], BF16)
            nc.sync.dma_start(wot_sb, wot)
            b_sb = {}
            for name, dram in (("bq", bq), ("bk", bk), ("bv", bv)):
                bt = singles.tile([CW, 1], F32, name=name + "_sb")
                nc.sync.dma_start(bt, dram)
                b_sb[name] = bt

            qm = acts.tile([CW, T], BF16)  # feature-major projections
            km = acts.tile([CW, T], BF16)
            vma = acts.tile([128, NBLK, 132], BF16)  # [t%128, blk, (hv|one|pad)x2]
            ctxn = acts.tile([CW, T], BF16)

            # ones columns of the augmented V (col 64 per head group)
            nc.vector.memset(
                vma.rearrange("p t (g c) -> p t g c", c=66)[:, :, :, 64], 1.0
            )

            # input tiles [128, DC, 512-tokens]: fine-grained streaming so the
            # attention pipeline can start as soon as the first k/q/v tiles land
            NTI = L // 512  # 4 token tiles per tensor per batch
            xt = {}
            dma_order = (
                [(0, "wk", 0), (0, "wq", 0), (0, "wv", 0), (0, "wk", 1),
                 (0, "wv", 1), (0, "wk", 2), (0, "wv", 2), (0, "wk", 3),
                 (0, "wv", 3), (0, "wq", 1), (0, "wq", 2), (0, "wq", 3),
                 (1, "wk", 0), (1, "wv", 0), (1, "wq", 0), (1, "wk", 1),
                 (1, "wv", 1), (1, "wk", 2), (1, "wv", 2), (1, "wk", 3),
                 (1, "wv", 3), (1, "wq", 1), (1, "wq", 2), (1, "wq", 3)]
            )
            dram_of = {"wk": kT, "wq": qT, "wv": vT}
            for (b, name, ti) in dma_order:
                t = slab_pool.tile(
                    [128, DC, 512], BF16, tag="xt", name=f"xt_{b}_{name}_{ti}"
                )
                nc.sync.dma_start(
                    t.rearrange("p c t -> p (c t)"),
                    dram_of[name][b * NTI + ti],
                )
                xt[(b, name, ti)] = t

            vmFs = {}

            def emit_proj(b, name, ti):
                """Project one 512-token tile for (batch, tensor)."""
                dstf = {"wq": qm, "wk": km}.get(name)
                if dstf is None and b not in vmFs:
                    vmFs[b] = vmf_pool.tile([128, L], F32, tag="vmF", name=f"vmF{b}")
                ps = pp_w.tile([128, 512], F32, tag="w", name="ps")
                for dc in range(DC):
                    nc.tensor.matmul(
                        ps,
                        lhsT=w_sb[name][:, dc, :],
                        rhs=xt[(b, name, ti)][:, dc, :],
                        start=(dc == 0),
                        stop=(dc == DC - 1),
                    )
                t0 = ti * 512
                dst = (
                    dstf[:, b * L + t0 : b * L + t0 + 512]
                    if dstf is not None
                    else vmFs[b][:, t0 : t0 + 512]
                )
                nc.vector.tensor_scalar_add(dst, ps, b_sb["b" + name[1]])

            def emit_vtrans(b, j):
                """Transpose 4 blocks of vm into token-major vma."""
                vmF = vmFs[b]
                tp = pp_w.tile([128, 512], F32, tag="w", name="tp")
                for i in range(4):
                    blk = j * 4 + i
                    nc.tensor.transpose(
                        tp[:, i * 128 : (i + 1) * 128],
                        vmF[:, blk * 128 : (blk + 1) * 128],
                        ident,
                    )
                nc.vector.tensor_copy(
                    vma.rearrange("p t (g c) -> p t g c", c=66)[
                        :,
                        b * (L // 128) + j * 4 : b * (L // 128) + j * 4 + 4,
                        :,
                        0:64,
                    ],
                    tp.rearrange("p (i g c) -> p i g c", i=4, g=2),
                )

            # ---- attention, flat software pipeline per batch ----
            NCHUNK = L // 512  # 4 query chunks per batch
            NPAIR = L // 256  # 8 key-tile pairs per chunk

            def emit_scores(b, c, p, state):
                ls = slice(b * L + c * 512, b * L + (c + 1) * 512)
                sp = [
                    pp_sp.tile([128, 1024], F32, tag="sp", name=f"sp{h}")
                    for h in range(2)
                ]
                for i in range(2):
                    pt = p * 2 + i
                    ks = slice(b * L + pt * 128, b * L + (pt + 1) * 128)
                    for h in range(2):
                        hs = slice(h * 64, (h + 1) * 64)
                        nc.tensor.matmul(
                            sp[h][:, i * 512 : (i + 1) * 512],
                            lhsT=km[hs, ks],
                            rhs=qm[hs, ls],
                            tile_position=(h * 64, 0),
                        )
                es = [
                    es_pool.tile([128, 1024], BF16, tag="es", name=f"es{h}")
                    for h in range(2)
                ]
                for h in range(2):
                    nc.scalar.activation(
                        es[h], sp[h], mybir.ActivationFunctionType.Exp, scale=SCALE
                    )
                state["es"][(b, c, p)] = es

            def emit_ctx(b, c, p, state):
                es = state["es"].pop((b, c, p))
                if p == 0:
                    state["ctx"][(b, c)] = [
                        pp_ctx.tile([128, 512], F32, tag="ctx", name=f"ctx{h}")
                        for h in range(2)
                    ]
                ctx = state["ctx"][(b, c)]
                for i in range(2):
                    pt = p * 2 + i
                    ptg = b * (L // 128) + pt
                    for h in range(2):
                        nc.tensor.matmul(
                            ctx[h][0:65, :],
                            lhsT=vma[:, ptg, h * 66 : h * 66 + 65],
                            rhs=es[h][:, i * 512 : (i + 1) * 512],
                            start=(p == 0 and i == 0),
                            stop=(p == NPAIR - 1 and i == 1),
                        )

            def emit_recip(b, c, state):
                # stage denominator rows to sbuf and ship them to the host,
                # which does the per-head division after the out-projection
                # (division by a head's denominator commutes with that
                # head's slice of Wo)
                ctx = state["ctx"][(b, c)]
                for h in range(2):
                    dsb = small.tile([1, 512], F32, tag=f"den{h}", name="dsb")
                    nc.vector.tensor_copy(dsb, ctx[h][64:65, :])
                    nc.sync.dma_start(
                        den_d[(b * (L // 512) + c) * 2 + h], dsb
                    )

            def emit_norm(b, c, state):
                ctx = state["ctx"][(b, c)]
                ls = slice(b * L + c * 512, b * L + (c + 1) * 512)
                for h in range(2):
                    nc.vector.tensor_copy(
                        ctxn[h * 64 : (h + 1) * 64, ls], ctx[h][0:64, :]
                    )

            def emit_outproj(b, c, half):
                for tt in (0, 1) if half == 0 else (2, 3):
                    t0 = b * L + c * 512 + tt * 128
                    for eh in range(2):
                        # per-head pair: concurrent row-packed K=64 matmuls
                        # into the two "w" psum banks
                        pos = [
                            pp_w.tile([128, 512], F32, tag="w", name=f"po{h}")
                            for h in range(2)
                        ]
                        for h in range(2):
                            nc.tensor.matmul(
                                pos[h],
                                lhsT=ctxn[h * 64 : (h + 1) * 64, t0 : t0 + 128],
                                rhs=wot_sb[
                                    h * 64 : (h + 1) * 64,
                                    eh * 512 : (eh + 1) * 512,
                                ],
                                tile_position=(h * 64, 0),
                            )
                        ob = ob_pool.tile([128, 1024], BF16, tag="ob", name="ob")
                        last = (b, c) == (B - 1, L // 512 - 1)
                        for h, od in ((0, out_p0), (1, out_p1)):
                            obh = ob[:, h * 512 : (h + 1) * 512]
                            if last and h == 1:
                                nc.scalar.copy(obh, pos[h])
                            else:
                                nc.vector.tensor_copy(obh, pos[h])
                            nc.sync.dma_start(
                                od[t0 : t0 + 128, eh * 512 : (eh + 1) * 512],
                                obh,
                            )

            # filler schedule over the single merged 64-step pipeline:
            # batch-0's remaining projections stream through its first steps
            # (need-ordered); batch-1's whole projection streams just-in-time
            # through the rest of batch-0's window
            V = "vtrans"
            fill = {
                1: [(0, "wv", 0), (0, V, 0), (0, "wk", 1), (0, "wv", 1),
                    (0, V, 1), (0, "wk", 2)],
                2: [(0, "wv", 2), (0, V, 2)],
                3: [(0, "wk", 3), (0, "wv", 3)],
                4: [(0, V, 3), (0, "wq", 1)],
                5: [(0, "wq", 2)],
                6: [(0, "wq", 3)],
                10: [(1, "wk", 0)], 12: [(1, "wv", 0)], 14: [(1, V, 0)],
                16: [(1, "wq", 0)], 18: [(1, "wk", 1)], 20: [(1, "wv", 1)],
                22: [(1, V, 1)], 24: [(1, "wk", 2)], 26: [(1, "wv", 2)],
                28: [(1, V, 2)], 30: [(1, "wk", 3)], 31: [(1, "wv", 3)],
                33: [(1, V, 3)], 35: [(1, "wq", 1)], 43: [(1, "wq", 2)],
                51: [(1, "wq", 3)],
            }

            for name, ti in (("wk", 0), ("wq", 0)):
                emit_proj(0, name, ti)

            NP_TOT = B * NCHUNK * NPAIR  # 64 pipeline steps

            def cp(pair):
                g, p = divmod(pair, NPAIR)
                return g // NCHUNK, g % NCHUNK, p  # batch, chunk, pair

            state = {"es": {}, "ctx": {}, "rc": {}}
            for s in range(NP_TOT + 5):
                # fillers first so downstream stages never wait on
                # later-emitted producers
                for (fb, kind, idx) in fill.pop(s, []):
                    if kind is V:
                        emit_vtrans(fb, idx)
                    else:
                        emit_proj(fb, kind, idx)
                if s < NP_TOT:
                    bb, c, p = cp(s)
                    emit_scores(bb, c, p, state)
                if 0 <= s - 2 < NP_TOT:
                    bb, c, p = cp(s - 2)
                    if p == NPAIR - 1:
                        emit_norm(bb, c, state)
                if 0 <= s - 1 < NP_TOT:
                    bb, c, p = cp(s - 1)
                    emit_ctx(bb, c, p, state)
                    if p == NPAIR - 1:
                        emit_recip(bb, c, state)
                for off, half in ((3, 0), (4, 1)):
                    if 0 <= s - off < NP_TOT:
                        bb, c, p = cp(s - off)
                        if p == NPAIR - 1:
                            emit_outproj(bb, c, half)
            assert not fill, f"unplaced fillers: {fill}"
    return _split_matmul_waits(nc)


_NC_CACHE = None


def build_in_maps(q, k, v, Wq, bq, Wk, bk, Wv, bv, Wo, bo):
    q, k, v = (np.asarray(x, np.float32) for x in (q, k, v))
    def retile(x):
        # [T, D] -> xT [D, T] -> [ntile=T//512, 128, DC*512] with each
        # (tile, partition) row contiguous
        xt = x.reshape(T, D).T.reshape(DC, 128, T // 512, 512)
        return np.ascontiguousarray(xt.transpose(2, 1, 0, 3)).reshape(
            T // 512, 128, DC * 512
        ).astype(bfloat16)

    qTh = retile(q)
    kTh = retile(k)
    vTh = retile(v)

    def swz(W, hs):
        # Wx.T column slice [D, CW] -> [128, DC*CW] so each sbuf partition's
        # row holds its DC weight chunks contiguously
        wt = np.asarray(W, np.float32).T[:, hs]
        return wt.reshape(DC, 128, CW).transpose(1, 0, 2).reshape(128, DC * CW).astype(bfloat16)

    c = np.ascontiguousarray
    in_maps = []
    for ci in range(N_CORES):
        hs = slice(ci * CW, (ci + 1) * CW)
        in_maps.append(
            {
                "qT": qTh,
                "kT": kTh,
                "vT": vTh,
                "wq": swz(Wq, hs),
                "wk": swz(Wk, hs),
                "wv": swz(Wv, hs),
                "wot": c(np.asarray(Wo, np.float32).T[hs, :]).astype(bfloat16),
                "bq": c(np.asarray(bq, np.float32)[hs, None]),
                "bk": c(np.asarray(bk, np.float32)[hs, None]),
                "bv": c(np.asarray(bv, np.float32)[hs, None]),
            }
        )
    return in_maps


def run(inputs, trace=False, **spmd_kwargs):
    global _NC_CACHE
    assert np.asarray(inputs["attention_mask"]).all(), "kernel assumes all-ones mask"
    if _NC_CACHE is None:
        _NC_CACHE = build_nc()
    nc = _NC_CACHE
    in_maps = build_in_maps(
        **{n: inputs[n] for n in ("q", "k", "v", "Wq", "bq", "Wk", "bk", "Wv", "bv", "Wo", "bo")}
    )
    res = bass_utils.run_bass_kernel_spmd(
        nc, in_maps, core_ids=list(range(N_CORES)), trace=trace, **spmd_kwargs
    )
    out = np.zeros((T, D), np.float32)
    for r in res.results:
        dd = np.asarray(r["den_d"], dtype=np.float32).reshape(-1, 2, 512)
        d0 = dd[:, 0, :].reshape(T)
        d1 = dd[:, 1, :].reshape(T)
        out += np.asarray(r["out_p0"], dtype=np.float32) / d0[:, None]
        out += np.asarray(r["out_p1"], dtype=np.float32) / d1[:, None]
    out += np.asarray(inputs["bo"], np.float32)[None, :]
    return out.reshape(B, L, D), res


def kernel(q, k, v, attention_mask, Wq, bq, Wk, bk, Wv, bv, Wo, bo):
    out, _ = run(dict(q=q, k=k, v=v, attention_mask=attention_mask, Wq=Wq, bq=bq,
                      Wk=Wk, bk=bk, Wv=Wv, bv=bv, Wo=Wo, bo=bo))
    return out
